# revision 29
# baseline (speedup 1.0000x reference)
"""Causal self-attention (B=4, T=2048, C=1024, H=16, D=64) on 8 trn2 NeuronCores.

Sharding: tensor-parallel over heads. Each core owns 2 heads:
  - computes Q^T/K^T/V for its heads from the (host-preblocked) full x^T,
  - causal attention (transposed-S, softmax denominator via an augmented
    ones-column on V),
  - partial output projection with its 128 rows of W_proj.
Host sums the 8 partial projections and adds (b_v @ W_proj + b_proj).

Key scheduling ideas:
  - AV phase is computed transposed (O^T = V^T-aug @ att^T) with V as the
    stationary operand and the exp'd score strips streamed at N<=512: the
    PE array streams MACs ~95% of the time instead of thrashing one
    LDWEIGHTS per 65-column matmul. O^T quarters are repaired to O via
    small PE transposes, which also yields the softmax denominators per
    query row for normalization.
  - The attention phase rate-limits on ACT exp, so independent matmul work
    is woven into the attention c-loops to keep the PE dense and its HAM
    clock gate at 2.4 GHz: att(b, h0) <- attT+projection of batch b-1;
    att(b, h1) <- QKV of batch b+1; last batch runs its own proj inline.
  - x^T and the weights are host-preblocked so every DMA line is >=2 KiB
    contiguous per partition (one descriptor per partition).
"""

from collections import deque

import numpy as np
import ml_dtypes

import concourse.bass as bass
import concourse.bacc as bacc
import concourse.mybir as mybir
import concourse.tile as tile

B, T, C, H, D = 4, 2048, 1024, 16, 64
NCORES = 8
HPC = H // NCORES  # heads per core = 2
P = 128
NB = T // P  # 16 blocks of 128 per sequence
CK = C // P  # 8 contraction chunks for the projections
NQ = B * 4  # 16 T/4-quarters across the batch

F32 = mybir.dt.float32
BF16 = mybir.dt.bfloat16
ADD = mybir.AluOpType.add
MULT = mybir.AluOpType.mult
EXP = mybir.ActivationFunctionType.Exp

# at_sb layout: strip for key-block c holds S^T blocks (c, j) for j in [c, NB),
# width (NB-c)*P, stored contiguously at OFF[c].
OFF = []
_cur = 0
for _c in range(NB):
    OFF.append(_cur)
    _cur += (NB - _c) * P
AT_W = _cur  # 136*128 = 17408


def attention_body(tc, outs, ins):
    nc = tc.nc
    xtb = ins["xtb"]  # [128, 16, 8, 512] bf16 host-preblocked x^T
    wq = ins["wq"]  # [128, 8, 128] bf16 preblocked (pre-scaled by 1/sqrt(D))
    wk = ins["wk"]  # [128, 8, 128] bf16 preblocked
    wv = ins["wv"]  # [128, 8, 128] bf16 preblocked
    wp = ins["wp"]          # [128, C] bf16
    bq = ins["bq"]          # [128, 1] f32 (pre-scaled by 1/sqrt(D))
    bk = ins["bk"]          # [128, 1] f32
    maskt = ins["maskt"]    # [128, 128] bf16: 1 if k<=q else 0 (multiplicative)
    ident = ins["ident"]    # [128, 128] bf16 identity
    out = outs["out"]       # [B*T, C] f32 partial projection output

    with (
        tc.tile_pool(name="consts", bufs=1) as consts,
        tc.tile_pool(name="xtp", bufs=4) as xtp,
        tc.tile_pool(name="qkp", bufs=2) as qkp,
        tc.tile_pool(name="vp", bufs=2) as vp,
        tc.tile_pool(name="atp", bufs=2) as atp,
        tc.tile_pool(name="smallp", bufs=4) as smallp,
        tc.tile_pool(name="outp", bufs=4) as outp,
        tc.tile_pool(name="psA", bufs=2, space="PSUM") as psA,
        tc.tile_pool(name="psAV", bufs=2, space="PSUM") as psAV,
        tc.tile_pool(name="psMM", bufs=2, space="PSUM") as psMM,
    ):
        # ---- constants (all linear DMAs); only prologue-critical ones are
        # issued here so the first x-quarter DMA isn't queued behind them.
        # wp/mask are issued after the prologue (first needed much later).
        wq_sb = consts.tile([P, CK, P], BF16, name="wq_sb")
        nc.sync.dma_start(wq_sb, wq)
        wk_sb = consts.tile([P, CK, P], BF16, name="wk_sb")
        nc.sync.dma_start(wk_sb, wk)
        wv_sb = consts.tile([P, CK, P], BF16, name="wv_sb")
        nc.sync.dma_start(wv_sb, wv)
        bq_sb = consts.tile([P, 1], F32, name="bq_sb")
        nc.sync.dma_start(bq_sb, bq)
        bk_sb = consts.tile([P, 1], F32, name="bk_sb")
        nc.sync.dma_start(bk_sb, bk)
        id_sb = consts.tile([P, P], BF16, name="id_sb")
        nc.sync.dma_start(id_sb, ident)
        wp_bf = consts.tile([P, C], BF16, name="wp_bf")
        mask_sb = consts.tile([P, P], BF16, name="mask_sb")

        qkv_tiles = {}
        attj_tiles = {}

        def make_qkv_items(b):
            """Work items (closures) that compute qt/kt/vaug for batch b."""
            qt = qkp.tile([P, T], BF16, tag="qt", name=f"qt_{b}")
            kt = qkp.tile([P, T], BF16, tag="kt", name=f"kt_{b}")
            vaug = vp.tile([P, NB, HPC, D + 1], BF16, tag="vaug", name=f"vaug_{b}")
            qkv_tiles[b] = (qt, kt, vaug)
            items = []
            xqs = {}

            def dma_item(q4):
                def go():
                    xq = xtp.tile([P, CK, 512], BF16, tag="xq", name=f"xq_{b}_{q4}")
                    xqs[q4] = xq
                    nc.sync.dma_start(xq, xtb[:, b * 4 + q4])
                return go

            def memset_item():
                def go():
                    nc.gpsimd.memset(vaug[:, :, :, D:], 1.0)
                return go

            def proj_item(q4, which):
                def go():
                    lo = q4 * 512
                    xq = xqs[q4]
                    w_sb, dst, bias = {
                        "q": (wq_sb, qt, bq_sb),
                        "k": (wk_sb, kt, bk_sb),
                    }.get(which, (wv_sb, None, None))
                    ps = psMM.tile([P, 512], F32, tag="mm", name=f"ps{which}_{b}_{q4}")
                    for cc in range(CK):
                        nc.tensor.matmul(
                            ps, lhsT=w_sb[:, cc], rhs=xq[:, cc],
                            start=(cc == 0), stop=(cc == CK - 1),
                        )
                    if which == "v":
                        vt = vp.tile([P, 512], BF16, tag="vt", name=f"vt_{b}_{q4}")
                        xqs[(q4, "vt")] = vt
                        nc.vector.tensor_copy(vt, ps)
                    else:
                        nc.vector.tensor_scalar(
                            dst[:, lo : lo + 512], ps, bias, None, ADD
                        )
                return go

            def vtr_item(q4, h):
                def go():
                    vt = xqs[(q4, "vt")]
                    # one accumulation group per head: mixing row-base 0/64
                    # transposes in one PSUM group faults on hardware
                    vtp = psMM.tile([P, 4, D], BF16, tag="mm", name=f"vtp_{b}_{q4}_{h}")
                    for t4 in range(4):
                        nc.tensor.matmul(
                            vtp[:, t4],
                            lhsT=vt[h * D : (h + 1) * D, t4 * P : (t4 + 1) * P],
                            rhs=id_sb[h * D : (h + 1) * D, h * D : (h + 1) * D],
                            is_transpose=True,
                            start=(t4 == 0), stop=(t4 == 3),
                        )
                    nc.vector.tensor_copy(vaug[:, q4 * 4 : (q4 + 1) * 4, h, 0:D], vtp)
                return go

            items.append(dma_item(0))
            items.append(memset_item())
            for q4 in range(4):
                items.append(proj_item(q4, "q"))
                if q4 + 1 < 4:
                    items.append(dma_item(q4 + 1))
                items.append(proj_item(q4, "k"))
                items.append(proj_item(q4, "v"))
                items.append(vtr_item(q4, 0))
                items.append(vtr_item(q4, 1))
            return deque(items)

        def proj_one(b, j, att_j, attT):
            """attT transpose + partial projection + DMA out for query block j."""
            tps = psMM.tile([P, P], BF16, tag="mm", name=f"tps_{b}_{j}")
            nc.tensor.matmul(
                tps, lhsT=att_j[j], rhs=id_sb,
                is_transpose=True, start=True, stop=True,
            )
            nc.vector.tensor_copy(attT[:, j * P : (j + 1) * P], tps)
            outst = outp.tile([P, C], BF16, tag="outst", name=f"outst_{b}_{j}")
            for n2 in range(2):
                pps = psMM.tile([P, 512], F32, tag="mm", name=f"pps_{b}_{j}_{n2}")
                nc.tensor.matmul(
                    pps,
                    lhsT=attT[:, j * P : (j + 1) * P],
                    rhs=wp_bf[:, n2 * 512 : (n2 + 1) * 512],
                    start=True, stop=True,
                )
                nc.any.tensor_copy(outst[:, n2 * 512 : (n2 + 1) * 512], pps)
            nc.sync.dma_start(out[b * T + j * P : b * T + (j + 1) * P, :], outst)

        def make_proj_items(b):
            att_j = attj_tiles.pop(b)
            attT = qkp.tile([P, T], BF16, tag="attT", name=f"attT_{b}")
            return deque(
                [(lambda j: (lambda: proj_one(b, j, att_j, attT)))(j) for j in range(NB)]
            )

        # prologue: QKV for batch 0; deferred const DMAs go right after the
        # first x-quarter fetch so they overlap the first QKV matmul groups
        prologue = make_qkv_items(0)
        prologue[0]()  # dma xq(0)
        nc.sync.dma_start(wp_bf, wp)
        nc.sync.dma_start(mask_sb, maskt)
        for it in list(prologue)[1:]:
            it()

        for b in range(B):
            qt, kt, vaug = qkv_tiles.pop(b)
            att_j = [
                smallp.tile([P, HPC * D], BF16, tag="attj", bufs=36, name=f"attj_{b}_{j}")
                for j in range(NB)
            ]
            attj_tiles[b] = att_j
            last = b == B - 1
            if last:
                attT_own = qkp.tile([P, T], BF16, tag="attT", name=f"attT_{b}")

            # fill work woven into this batch's attention steps:
            #   h0 steps <- projection of batch b-1; h1 steps <- QKV of batch b+1
            fill = {0: deque(), 1: deque()}
            if b > 0:
                fill[0] = make_proj_items(b - 1)
            if b + 1 < B:
                fill[1] = make_qkv_items(b + 1)
            if b == 0:
                q = fill[1]
                fill[0] = deque(list(q)[: len(q) // 2])
                fill[1] = deque(list(q)[len(q) // 2 :])

            for h in range(HPC):
                hs = h * D
                at_sb = atp.tile([P, AT_W], BF16, tag="at", name=f"at_{b}_{h}")
                fq = fill[h]
                n_total = len(fq)
                popped = 0
                for c in range(NB):
                    w = (NB - c) * P
                    lhs_k = kt[hs : hs + D, c * P : (c + 1) * P]
                    # strip pieces of <=1024 cols (2 PSUM banks)
                    po = 0
                    while po < w:
                        pw = min(1024, w - po)
                        sps = psA.tile([P, 1024], F32, tag="sA", name=f"sps_{b}_{h}_{c}_{po}")
                        col = 0
                        while col < pw:
                            n = min(512, pw - col)
                            nc.tensor.matmul(
                                sps[:, col : col + n],
                                lhsT=lhs_k,
                                rhs=qt[hs : hs + D, c * P + po + col : c * P + po + col + n],
                                start=True, stop=True,
                            )
                            col += n
                        nc.scalar.activation(
                            at_sb[:, OFF[c] + po : OFF[c] + po + pw], sps[:, :pw], EXP
                        )
                        po += pw
                    # diagonal block: multiplicative causal mask (GpSimd, SBUF)
                    nc.gpsimd.tensor_tensor(
                        at_sb[:, OFF[c] : OFF[c] + P],
                        at_sb[:, OFF[c] : OFF[c] + P],
                        mask_sb,
                        MULT,
                    )
                    # AV for a full query quarter once its strips are exp'd:
                    # O^T[d_aug, q] = sum_c vaug[:,c,h].T @ at-strip(c) -- V is
                    # the stationary operand, score strips stream at N<=512.
                    if (c + 1) % 4 == 0:
                        j4 = c // 4
                        top = 4 * j4 + 3
                        otp = psAV.tile([D + 1, 512], F32, tag="av", name=f"otp_{b}_{h}_{j4}")
                        for c2 in range(top + 1):
                            if c2 <= 4 * j4:
                                nc.tensor.matmul(
                                    otp,
                                    lhsT=vaug[:, c2, h],
                                    rhs=at_sb[:, OFF[c2] + (4 * j4 - c2) * P : OFF[c2] + (4 * j4 - c2) * P + 512],
                                    start=(c2 == 0), stop=(c2 == top),
                                )
                            else:
                                nw = (top + 1 - c2) * P
                                nc.tensor.matmul(
                                    otp[:, 512 - nw :],
                                    lhsT=vaug[:, c2, h],
                                    rhs=at_sb[:, OFF[c2] : OFF[c2] + nw],
                                    start=False, stop=(c2 == top),
                                )
                        ot_sb = smallp.tile([D + 1, 512], BF16, tag="ot", bufs=4, name=f"ot_{b}_{h}_{j4}")
                        nc.vector.tensor_copy(ot_sb, otp)
                        # repair: transpose each 128-q block back to [q, d_aug],
                        # normalize rows by the denominator column
                        for jj in range(4):
                            j = 4 * j4 + jj
                            aps = psMM.tile([P, D + 1], BF16, tag="mm", name=f"aps_{b}_{h}_{j}")
                            nc.tensor.matmul(
                                aps,
                                lhsT=ot_sb[:, jj * P : (jj + 1) * P],
                                rhs=id_sb[0 : D + 1, 0 : D + 1],
                                is_transpose=True, start=True, stop=True,
                            )
                            r = smallp.tile([P, 1], F32, tag="r", name=f"r_{b}_{h}_{j}")
                            nc.vector.reciprocal(r, aps[:, D : D + 1])
                            nc.vector.tensor_scalar(
                                att_j[j][:, hs : hs + D], aps[:, 0:D], r, None, MULT
                            )
                            if last and h == HPC - 1:
                                # defer own proj to the following light steps
                                fq.append(
                                    (lambda jc: (lambda: proj_one(b, jc, att_j, attT_own)))(j)
                                )
                    # weave in fill work, front-loaded onto the light steps
                    if last and h == HPC - 1:
                        # steady drain: own proj items arrive at chunk ends
                        for _ in range(2):
                            if fq:
                                fq.popleft()()
                    else:
                        if c == NB - 1:
                            want = n_total
                        else:
                            want = min(n_total, ((c + 1) * n_total + 11) // 12)
                        while popped < want:
                            fq.popleft()()
                            popped += 1
                # drain any remaining fill (last batch's final proj items)
                while fq:
                    fq.popleft()()


def build_nc():
    nc = bacc.Bacc("TRN2", debug=False, enable_asserts=False, num_devices=NCORES)
    ins = {
        "xtb": nc.dram_tensor("xtb", [P, NQ, CK, 512], BF16, kind="ExternalInput").ap(),
        "wq": nc.dram_tensor("wq", [P, CK, P], BF16, kind="ExternalInput").ap(),
        "wk": nc.dram_tensor("wk", [P, CK, P], BF16, kind="ExternalInput").ap(),
        "wv": nc.dram_tensor("wv", [P, CK, P], BF16, kind="ExternalInput").ap(),
        "wp": nc.dram_tensor("wp", [P, C], BF16, kind="ExternalInput").ap(),
        "bq": nc.dram_tensor("bq", [P, 1], F32, kind="ExternalInput").ap(),
        "bk": nc.dram_tensor("bk", [P, 1], F32, kind="ExternalInput").ap(),
        "maskt": nc.dram_tensor("maskt", [P, P], BF16, kind="ExternalInput").ap(),
        "ident": nc.dram_tensor("ident", [P, P], BF16, kind="ExternalInput").ap(),
    }
    outs = {"out": nc.dram_tensor("out", [B * T, C], BF16, kind="ExternalOutput").ap()}
    with tile.TileContext(nc) as tc:
        attention_body(tc, outs, ins)
    nc.compile()
    return nc


def _preblock_w(w):
    # [C, 128] -> [128, CK, 128]: partition-major blocks for linear DMA
    return np.ascontiguousarray(
        w.reshape(CK, P, P).transpose(1, 0, 2)
    ).astype(ml_dtypes.bfloat16)


def make_in_maps(inputs, W_qkv, b_qkv, W_proj):
    x2 = np.asarray(inputs, np.float32).reshape(B * T, C)
    xtv = np.ascontiguousarray(x2.T)  # [C, B*T]
    # preblock: [C, B*T] -> [128, 16, 8, 512]
    xtb = np.ascontiguousarray(
        xtv.reshape(CK, P, NQ, 512).transpose(1, 2, 0, 3)
    ).astype(ml_dtypes.bfloat16)
    W_qkv = np.asarray(W_qkv, np.float32)
    b_qkv = np.asarray(b_qkv, np.float32)
    W_proj = np.asarray(W_proj, np.float32)
    identv = np.eye(P, dtype=ml_dtypes.bfloat16)
    masktv = np.triu(np.ones((P, P), np.float32)).astype(ml_dtypes.bfloat16)
    in_maps = []
    for cid in range(NCORES):
        s = cid * HPC * D
        in_maps.append({
            "xtb": xtb,
            "wq": _preblock_w(W_qkv[:, s : s + P] * 0.125),
            "wk": _preblock_w(W_qkv[:, C + s : C + s + P]),
            "wv": _preblock_w(W_qkv[:, 2 * C + s : 2 * C + s + P]),
            "wp": np.ascontiguousarray(W_proj[s : s + P, :]).astype(ml_dtypes.bfloat16),
            "bq": np.ascontiguousarray(b_qkv[s : s + P].reshape(P, 1) * 0.125),
            "bk": np.ascontiguousarray(b_qkv[C + s : C + s + P].reshape(P, 1)),
            "maskt": masktv,
            "ident": identv,
        })
    return in_maps


_NC_CACHE = {}


def run(inputs, W_qkv, b_qkv, W_proj, b_proj, trace=False, **kw):
    from concourse.bass_utils import run_bass_kernel_spmd

    if "nc" not in _NC_CACHE:
        _NC_CACHE["nc"] = build_nc()
    nc = _NC_CACHE["nc"]
    in_maps = make_in_maps(inputs, W_qkv, b_qkv, W_proj)
    res = run_bass_kernel_spmd(nc, in_maps, core_ids=list(range(NCORES)), trace=trace, **kw)
    acc = res.results[0]["out"].astype(np.float32)
    for cid in range(1, NCORES):
        acc += res.results[cid]["out"].astype(np.float32)
    host_bias = np.asarray(b_qkv, np.float32)[2 * C :] @ np.asarray(W_proj, np.float32)
    host_bias = host_bias + np.asarray(b_proj, np.float32)
    outv = (acc + host_bias[None, :]).reshape(B, T, C).astype(np.float32)
    return outv, res


def kernel(inputs, W_qkv, b_qkv, W_proj, b_proj):
    outv, _ = run(inputs, W_qkv, b_qkv, W_proj, b_proj, trace=False)
    return outv


# revision 30
# speedup vs baseline: 1.0515x; 1.0515x over previous
"""Causal self-attention (B=4, T=2048, C=1024, H=16, D=64) on 8 trn2 NeuronCores.

Sharding: tensor-parallel over heads. Each core owns 2 heads:
  - computes Q^T/K^T/V for its heads from the (host-preblocked) full x^T,
  - causal attention (transposed-S, softmax denominator via an augmented
    ones-column on V),
  - partial output projection with its 128 rows of W_proj.
Host sums the 8 partial projections and adds (b_v @ W_proj + b_proj).

Key scheduling ideas:
  - AV phase is computed transposed (O^T = V^T-aug @ att^T) with V as the
    stationary operand and the exp'd score strips streamed at N<=512: the
    PE array streams MACs ~95% of the time instead of thrashing one
    LDWEIGHTS per 65-column matmul. O^T quarters are repaired to O via
    small PE transposes, which also yields the softmax denominators per
    query row for normalization.
  - The attention phase rate-limits on ACT exp, so independent matmul work
    is woven into the attention c-loops to keep the PE dense and its HAM
    clock gate at 2.4 GHz: att(b, h0) <- attT+projection of batch b-1;
    att(b, h1) <- QKV of batch b+1; last batch runs its own proj inline.
  - x^T and the weights are host-preblocked so every DMA line is >=2 KiB
    contiguous per partition (one descriptor per partition).
"""

from collections import deque

import numpy as np
import ml_dtypes

import concourse.bass as bass
import concourse.bacc as bacc
import concourse.mybir as mybir
import concourse.tile as tile

B, T, C, H, D = 4, 2048, 1024, 16, 64
NCORES = 8
HPC = H // NCORES  # heads per core = 2
P = 128
NB = T // P  # 16 blocks of 128 per sequence
CK = C // P  # 8 contraction chunks for the projections
NQ = B * 4  # 16 T/4-quarters across the batch

F32 = mybir.dt.float32
BF16 = mybir.dt.bfloat16
ADD = mybir.AluOpType.add
MULT = mybir.AluOpType.mult
EXP = mybir.ActivationFunctionType.Exp

# at_sb layout: strip for key-block c holds S^T blocks (c, j) for j in [c, NB),
# width (NB-c)*P, stored contiguously at OFF[c].
OFF = []
_cur = 0
for _c in range(NB):
    OFF.append(_cur)
    _cur += (NB - _c) * P
AT_W = _cur  # 136*128 = 17408


def attention_body(tc, outs, ins):
    nc = tc.nc
    xtb = ins["xtb"]  # [128, 16, 8, 512] bf16 host-preblocked x^T
    wq = ins["wq"]  # [128, 8, 128] bf16 preblocked (pre-scaled by 1/sqrt(D))
    wk = ins["wk"]  # [128, 8, 128] bf16 preblocked
    wv = ins["wv"]  # [128, 8, 128] bf16 preblocked
    wp = ins["wp"]          # [128, C] bf16
    bq = ins["bq"]          # [128, 1] f32 (pre-scaled by 1/sqrt(D))
    bk = ins["bk"]          # [128, 1] f32
    maskt = ins["maskt"]    # [128, 128] bf16: 1 if k<=q else 0 (multiplicative)
    ident = ins["ident"]    # [128, 128] bf16 identity
    out = outs["out"]       # [B*T, C] f32 partial projection output

    with (
        tc.tile_pool(name="consts", bufs=1) as consts,
        tc.tile_pool(name="xtp", bufs=3) as xtp,
        tc.tile_pool(name="qkp", bufs=2) as qkp,
        tc.tile_pool(name="vp", bufs=2) as vp,
        tc.tile_pool(name="atp", bufs=2) as atp,
        tc.tile_pool(name="smallp", bufs=4) as smallp,
        tc.tile_pool(name="outp", bufs=3) as outp,
        tc.tile_pool(name="psA", bufs=2, space="PSUM") as psA,
        tc.tile_pool(name="psAV", bufs=2, space="PSUM") as psAV,
        tc.tile_pool(name="psMM", bufs=2, space="PSUM") as psMM,
    ):
        # ---- constants (all linear DMAs); only prologue-critical ones are
        # issued here so the first x-quarter DMA isn't queued behind them.
        # wp/mask are issued after the prologue (first needed much later).
        wq_sb = consts.tile([P, CK, P], BF16, name="wq_sb")
        nc.sync.dma_start(wq_sb, wq)
        wk_sb = consts.tile([P, CK, P], BF16, name="wk_sb")
        nc.sync.dma_start(wk_sb, wk)
        wv_sb = consts.tile([P, CK, P], BF16, name="wv_sb")
        nc.sync.dma_start(wv_sb, wv)
        bq_sb = consts.tile([P, 1], F32, name="bq_sb")
        nc.sync.dma_start(bq_sb, bq)
        bk_sb = consts.tile([P, 1], F32, name="bk_sb")
        nc.sync.dma_start(bk_sb, bk)
        id_sb = consts.tile([P, P], BF16, name="id_sb")
        nc.sync.dma_start(id_sb, ident)
        wp_bf = consts.tile([P, C], BF16, name="wp_bf")
        nc.sync.dma_start(wp_bf, wp)
        mask_sb = consts.tile([P, P], BF16, name="mask_sb")
        nc.sync.dma_start(mask_sb, maskt)

        qkv_tiles = {}
        attj_tiles = {}

        def make_qkv_items(b):
            """Work items (closures) that compute qt/kt/vaug for batch b."""
            qt = qkp.tile([P, T], BF16, tag="qt", name=f"qt_{b}")
            kt = qkp.tile([P, T], BF16, tag="kt", name=f"kt_{b}")
            vaug = vp.tile([P, NB, HPC, D + 1], BF16, tag="vaug", name=f"vaug_{b}")
            qkv_tiles[b] = (qt, kt, vaug)
            items = []
            xqs = {}

            def dma_item(q4):
                def go():
                    xq = xtp.tile([P, CK, 512], BF16, tag="xq", name=f"xq_{b}_{q4}")
                    xqs[q4] = xq
                    nc.sync.dma_start(xq, xtb[:, b * 4 + q4])
                return go

            def memset_item():
                def go():
                    nc.gpsimd.memset(vaug[:, :, :, D:], 1.0)
                return go

            def proj_item(q4, which):
                def go():
                    lo = q4 * 512
                    xq = xqs[q4]
                    w_sb, dst, bias = {
                        "q": (wq_sb, qt, bq_sb),
                        "k": (wk_sb, kt, bk_sb),
                    }.get(which, (wv_sb, None, None))
                    ps = psMM.tile([P, 512], F32, tag="mm", name=f"ps{which}_{b}_{q4}")
                    for cc in range(CK):
                        nc.tensor.matmul(
                            ps, lhsT=w_sb[:, cc], rhs=xq[:, cc],
                            start=(cc == 0), stop=(cc == CK - 1),
                        )
                    if which == "v":
                        vt = vp.tile([P, 512], BF16, tag="vt", name=f"vt_{b}_{q4}")
                        xqs[(q4, "vt")] = vt
                        nc.vector.tensor_copy(vt, ps)
                    else:
                        nc.vector.tensor_scalar(
                            dst[:, lo : lo + 512], ps, bias, None, ADD
                        )
                return go

            def vtr_item(q4, h):
                def go():
                    vt = xqs[(q4, "vt")]
                    # one accumulation group per head: mixing row-base 0/64
                    # transposes in one PSUM group faults on hardware
                    vtp = psMM.tile([P, 4, D], BF16, tag="mm", name=f"vtp_{b}_{q4}_{h}")
                    for t4 in range(4):
                        nc.tensor.matmul(
                            vtp[:, t4],
                            lhsT=vt[h * D : (h + 1) * D, t4 * P : (t4 + 1) * P],
                            rhs=id_sb[h * D : (h + 1) * D, h * D : (h + 1) * D],
                            is_transpose=True,
                            start=(t4 == 0), stop=(t4 == 3),
                        )
                    nc.vector.tensor_copy(vaug[:, q4 * 4 : (q4 + 1) * 4, h, 0:D], vtp)
                return go

            items.append(dma_item(0))
            items.append(memset_item())
            for q4 in range(4):
                items.append(proj_item(q4, "q"))
                if q4 + 1 < 4:
                    items.append(dma_item(q4 + 1))
                items.append(proj_item(q4, "k"))
                items.append(proj_item(q4, "v"))
                items.append(vtr_item(q4, 0))
                items.append(vtr_item(q4, 1))
            return deque(items)

        def proj_one(b, j, att_j, attT):
            """attT transpose + partial projection + DMA out for query block j."""
            tps = psMM.tile([P, P], BF16, tag="mm", name=f"tps_{b}_{j}")
            nc.tensor.matmul(
                tps, lhsT=att_j[j], rhs=id_sb,
                is_transpose=True, start=True, stop=True,
            )
            nc.vector.tensor_copy(attT[:, j * P : (j + 1) * P], tps)
            outst = outp.tile([P, C], BF16, tag="outst", name=f"outst_{b}_{j}")
            for n2 in range(2):
                pps = psMM.tile([P, 512], F32, tag="mm", name=f"pps_{b}_{j}_{n2}")
                nc.tensor.matmul(
                    pps,
                    lhsT=attT[:, j * P : (j + 1) * P],
                    rhs=wp_bf[:, n2 * 512 : (n2 + 1) * 512],
                    start=True, stop=True,
                )
                nc.any.tensor_copy(outst[:, n2 * 512 : (n2 + 1) * 512], pps)
            nc.sync.dma_start(out[b * T + j * P : b * T + (j + 1) * P, :], outst)

        def make_proj_items(b):
            att_j = attj_tiles.pop(b)
            attT = qkp.tile([P, T], BF16, tag="attT", name=f"attT_{b}")
            return deque(
                [(lambda j: (lambda: proj_one(b, j, att_j, attT)))(j) for j in range(NB)]
            )

        # prologue: QKV for batch 0
        for it in make_qkv_items(0):
            it()

        for b in range(B):
            qt, kt, vaug = qkv_tiles.pop(b)
            att_j = [
                smallp.tile([P, HPC * D], BF16, tag="attj", bufs=36, name=f"attj_{b}_{j}")
                for j in range(NB)
            ]
            attj_tiles[b] = att_j
            last = b == B - 1
            if last:
                attT_own = qkp.tile([P, T], BF16, tag="attT", name=f"attT_{b}")

            # fill work woven into this batch's attention steps:
            #   h0 steps <- projection of batch b-1; h1 steps <- QKV of batch b+1
            fill = {0: deque(), 1: deque()}
            if b > 0:
                fill[0] = make_proj_items(b - 1)
            if b + 1 < B:
                fill[1] = make_qkv_items(b + 1)
            if b == 0:
                q = fill[1]
                fill[0] = deque(list(q)[: len(q) // 2])
                fill[1] = deque(list(q)[len(q) // 2 :])

            for h in range(HPC):
                hs = h * D
                at_sb = atp.tile([P, AT_W], BF16, tag="at", name=f"at_{b}_{h}")
                fq = fill[h]
                n_total = len(fq)
                popped = 0
                for c in range(NB):
                    w = (NB - c) * P
                    lhs_k = kt[hs : hs + D, c * P : (c + 1) * P]
                    # strip pieces of <=1024 cols (2 PSUM banks)
                    po = 0
                    while po < w:
                        pw = min(1024, w - po)
                        sps = psA.tile([P, 1024], F32, tag="sA", name=f"sps_{b}_{h}_{c}_{po}")
                        col = 0
                        while col < pw:
                            n = min(512, pw - col)
                            nc.tensor.matmul(
                                sps[:, col : col + n],
                                lhsT=lhs_k,
                                rhs=qt[hs : hs + D, c * P + po + col : c * P + po + col + n],
                                start=True, stop=True,
                            )
                            col += n
                        nc.scalar.activation(
                            at_sb[:, OFF[c] + po : OFF[c] + po + pw], sps[:, :pw], EXP
                        )
                        po += pw
                    # diagonal block: multiplicative causal mask (GpSimd, SBUF)
                    nc.gpsimd.tensor_tensor(
                        at_sb[:, OFF[c] : OFF[c] + P],
                        at_sb[:, OFF[c] : OFF[c] + P],
                        mask_sb,
                        MULT,
                    )
                    # AV for a full query quarter once its strips are exp'd:
                    # O^T[d_aug, q] = sum_c vaug[:,c,h].T @ at-strip(c) -- V is
                    # the stationary operand, score strips stream at N<=512.
                    if (c + 1) % 4 == 0:
                        j4 = c // 4
                        top = 4 * j4 + 3
                        otp = psAV.tile([D + 1, 512], F32, tag="av", name=f"otp_{b}_{h}_{j4}")
                        for c2 in range(top + 1):
                            if c2 <= 4 * j4:
                                nc.tensor.matmul(
                                    otp,
                                    lhsT=vaug[:, c2, h],
                                    rhs=at_sb[:, OFF[c2] + (4 * j4 - c2) * P : OFF[c2] + (4 * j4 - c2) * P + 512],
                                    start=(c2 == 0), stop=(c2 == top),
                                )
                            else:
                                nw = (top + 1 - c2) * P
                                nc.tensor.matmul(
                                    otp[:, 512 - nw :],
                                    lhsT=vaug[:, c2, h],
                                    rhs=at_sb[:, OFF[c2] : OFF[c2] + nw],
                                    start=False, stop=(c2 == top),
                                )
                        ot_sb = smallp.tile([D + 1, 512], BF16, tag="ot", bufs=3, name=f"ot_{b}_{h}_{j4}")
                        nc.vector.tensor_copy(ot_sb, otp)
                        # repair: transpose each 128-q block back to [q, d_aug],
                        # normalize rows by the denominator column
                        for jj in range(4):
                            j = 4 * j4 + jj
                            aps = psMM.tile([P, D + 1], BF16, tag="mm", name=f"aps_{b}_{h}_{j}")
                            nc.tensor.matmul(
                                aps,
                                lhsT=ot_sb[:, jj * P : (jj + 1) * P],
                                rhs=id_sb[0 : D + 1, 0 : D + 1],
                                is_transpose=True, start=True, stop=True,
                            )
                            r = smallp.tile([P, 1], F32, tag="r", name=f"r_{b}_{h}_{j}")
                            nc.vector.reciprocal(r, aps[:, D : D + 1])
                            nc.vector.tensor_scalar(
                                att_j[j][:, hs : hs + D], aps[:, 0:D], r, None, MULT
                            )
                            if last and h == HPC - 1:
                                # defer own proj to the following light steps
                                fq.append(
                                    (lambda jc: (lambda: proj_one(b, jc, att_j, attT_own)))(j)
                                )
                    # weave in fill work, front-loaded onto the light steps
                    if last and h == HPC - 1:
                        # steady drain: own proj items arrive at chunk ends
                        for _ in range(2):
                            if fq:
                                fq.popleft()()
                    else:
                        if c == NB - 1:
                            want = n_total
                        else:
                            want = min(n_total, ((c + 1) * n_total + 11) // 12)
                        while popped < want:
                            fq.popleft()()
                            popped += 1
                # drain any remaining fill (last batch's final proj items)
                while fq:
                    fq.popleft()()


def build_nc():
    nc = bacc.Bacc("TRN2", debug=False, enable_asserts=False, num_devices=NCORES)
    ins = {
        "xtb": nc.dram_tensor("xtb", [P, NQ, CK, 512], BF16, kind="ExternalInput").ap(),
        "wq": nc.dram_tensor("wq", [P, CK, P], BF16, kind="ExternalInput").ap(),
        "wk": nc.dram_tensor("wk", [P, CK, P], BF16, kind="ExternalInput").ap(),
        "wv": nc.dram_tensor("wv", [P, CK, P], BF16, kind="ExternalInput").ap(),
        "wp": nc.dram_tensor("wp", [P, C], BF16, kind="ExternalInput").ap(),
        "bq": nc.dram_tensor("bq", [P, 1], F32, kind="ExternalInput").ap(),
        "bk": nc.dram_tensor("bk", [P, 1], F32, kind="ExternalInput").ap(),
        "maskt": nc.dram_tensor("maskt", [P, P], BF16, kind="ExternalInput").ap(),
        "ident": nc.dram_tensor("ident", [P, P], BF16, kind="ExternalInput").ap(),
    }
    outs = {"out": nc.dram_tensor("out", [B * T, C], BF16, kind="ExternalOutput").ap()}
    with tile.TileContext(nc) as tc:
        attention_body(tc, outs, ins)
    nc.compile()
    return nc


def _preblock_w(w):
    # [C, 128] -> [128, CK, 128]: partition-major blocks for linear DMA
    return np.ascontiguousarray(
        w.reshape(CK, P, P).transpose(1, 0, 2)
    ).astype(ml_dtypes.bfloat16)


def make_in_maps(inputs, W_qkv, b_qkv, W_proj):
    x2 = np.asarray(inputs, np.float32).reshape(B * T, C)
    xtv = np.ascontiguousarray(x2.T)  # [C, B*T]
    # preblock: [C, B*T] -> [128, 16, 8, 512]
    xtb = np.ascontiguousarray(
        xtv.reshape(CK, P, NQ, 512).transpose(1, 2, 0, 3)
    ).astype(ml_dtypes.bfloat16)
    W_qkv = np.asarray(W_qkv, np.float32)
    b_qkv = np.asarray(b_qkv, np.float32)
    W_proj = np.asarray(W_proj, np.float32)
    identv = np.eye(P, dtype=ml_dtypes.bfloat16)
    masktv = np.triu(np.ones((P, P), np.float32)).astype(ml_dtypes.bfloat16)
    in_maps = []
    for cid in range(NCORES):
        s = cid * HPC * D
        in_maps.append({
            "xtb": xtb,
            "wq": _preblock_w(W_qkv[:, s : s + P] * 0.125),
            "wk": _preblock_w(W_qkv[:, C + s : C + s + P]),
            "wv": _preblock_w(W_qkv[:, 2 * C + s : 2 * C + s + P]),
            "wp": np.ascontiguousarray(W_proj[s : s + P, :]).astype(ml_dtypes.bfloat16),
            "bq": np.ascontiguousarray(b_qkv[s : s + P].reshape(P, 1) * 0.125),
            "bk": np.ascontiguousarray(b_qkv[C + s : C + s + P].reshape(P, 1)),
            "maskt": masktv,
            "ident": identv,
        })
    return in_maps


_NC_CACHE = {}


def run(inputs, W_qkv, b_qkv, W_proj, b_proj, trace=False, **kw):
    from concourse.bass_utils import run_bass_kernel_spmd

    if "nc" not in _NC_CACHE:
        _NC_CACHE["nc"] = build_nc()
    nc = _NC_CACHE["nc"]
    in_maps = make_in_maps(inputs, W_qkv, b_qkv, W_proj)
    res = run_bass_kernel_spmd(nc, in_maps, core_ids=list(range(NCORES)), trace=trace, **kw)
    acc = res.results[0]["out"].astype(np.float32)
    for cid in range(1, NCORES):
        acc += res.results[cid]["out"].astype(np.float32)
    host_bias = np.asarray(b_qkv, np.float32)[2 * C :] @ np.asarray(W_proj, np.float32)
    host_bias = host_bias + np.asarray(b_proj, np.float32)
    outv = (acc + host_bias[None, :]).reshape(B, T, C).astype(np.float32)
    return outv, res


def kernel(inputs, W_qkv, b_qkv, W_proj, b_proj):
    outv, _ = run(inputs, W_qkv, b_qkv, W_proj, b_proj, trace=False)
    return outv
